# revision 2
# baseline (speedup 1.0000x reference)
"""Trainium2 Bass kernel for nn_ClassBasedSMDecoder.

Reference computation (N=8192 tokens, H=1024 hid, C=1024 classes, K=32):
    p_class = x @ W_cls.T + b_cls                      # [N, C]
    p_words = einsum('nh,nkh->nk', x, W_words[cls]) + b_words[cls]   # [N, K]

Sharding over 8 NeuronCores:
  * p_class: data-parallel over tokens — core i computes tokens
    [i*1024, (i+1)*1024) against the full (replicated) W_cls.
  * p_words: expert-parallel — core i owns classes [i*128, (i+1)*128).
    The host routes tokens to their class's core (sorted by class and
    padded to a fixed per-class capacity), each core runs per-class
    [cap, H] @ [H, K] matmuls with 4 classes packed into the 128-wide
    PE array via column tiling, and the host scatters results back.

All matmul inputs are cast to bf16 on the host (fp32 accumulate in PSUM).
"""

import numpy as np
import ml_dtypes

import concourse.bass as bass
import concourse.mybir as mybir
import concourse.tile as tile
from concourse import bacc
from concourse.bass_utils import run_bass_kernel_spmd

BF16 = ml_dtypes.bfloat16

N, H, C, K = 8192, 1024, 1024, 32
NCORES = 8
CS = C // NCORES        # 128 classes per core
TOK = N // NCORES       # 1024 tokens per core (p_class shard)
HC = H // 128           # 8 contraction chunks
NGRP = CS // 4          # 32 groups of 4 classes (column tiling)

_cache: dict = {}


def _build(cap: int):
    """Build + compile the per-core Bass program for class capacity `cap`."""
    G = CS * cap  # routed-token columns per core (padded)
    dt = mybir.dt
    nc = bacc.Bacc(
        "TRN2", target_bir_lowering=False, debug=False, enable_asserts=False
    )

    xt = nc.dram_tensor("xt", [HC, 128, TOK], dt.bfloat16, kind="ExternalInput")
    wct = nc.dram_tensor("wct", [HC, 128, C], dt.bfloat16, kind="ExternalInput")
    wwt = nc.dram_tensor("wwt", [HC, 128, CS * K], dt.bfloat16, kind="ExternalInput")
    xgt = nc.dram_tensor("xgt", [HC, 128, G], dt.bfloat16, kind="ExternalInput")
    pc = nc.dram_tensor("pc", [TOK, C], dt.float32, kind="ExternalOutput")
    pw = nc.dram_tensor("pw", [128, NGRP * cap], dt.float32, kind="ExternalOutput")

    with tile.TileContext(nc) as tc:
        with (
            tc.tile_pool(name="big", bufs=1) as big,
            tc.tile_pool(name="stage", bufs=4) as stage,
            tc.tile_pool(name="ps_pc", bufs=4, space=bass.MemorySpace.PSUM) as ps_pc,
            tc.tile_pool(name="ps_pw", bufs=4, space=bass.MemorySpace.PSUM) as ps_pw,
        ):
            xt_s = [big.tile([128, TOK], dt.bfloat16, name=f"xt{h}", tag=f"xt{h}") for h in range(HC)]
            wct_s = [big.tile([128, C], dt.bfloat16, name=f"wct{h}", tag=f"wct{h}") for h in range(HC)]
            wwt_s = [
                big.tile([128, CS * K], dt.bfloat16, name=f"wwt{h}", tag=f"wwt{h}") for h in range(HC)
            ]
            xgt_s = [big.tile([128, G], dt.bfloat16, name=f"xgt{h}", tag=f"xgt{h}") for h in range(HC)]

            # Loads needed first by p_class, then the p_words operands.
            for h in range(HC):
                nc.sync.dma_start(xt_s[h][:], xt[h])
                nc.sync.dma_start(wct_s[h][:], wct[h])
            for h in range(HC):
                nc.sync.dma_start(wwt_s[h][:], wwt[h])
                nc.sync.dma_start(xgt_s[h][:], xgt[h])

            # ---- p_class: out[tok, cls] = sum_h x.T[h, tok] * W_cls.T[h, cls]
            for mt in range(TOK // 128):       # 8 token tiles (stationary)
                for ct in range(C // 512):     # 2 class column tiles (moving)
                    acc = ps_pc.tile([128, 512], dt.float32, tag="pcacc")
                    for h in range(HC):
                        nc.tensor.matmul(
                            acc[:],
                            xt_s[h][:, mt * 128:(mt + 1) * 128],
                            wct_s[h][:, ct * 512:(ct + 1) * 512],
                            start=(h == 0),
                            stop=(h == HC - 1),
                        )
                    st = stage.tile([128, 512], dt.float32, tag="pcst")
                    nc.vector.tensor_copy(st[:], acc[:])
                    nc.scalar.dma_start(
                        pc[mt * 128:(mt + 1) * 128, ct * 512:(ct + 1) * 512], st[:]
                    )

            # ---- p_words: per class c, out[w, t] = sum_h Ww.T[h, c, w] * xg.T[h, t]
            # 4 classes per PE pass via column tiling (32-col groups).
            pwst = big.tile([128, NGRP * cap], dt.float32, tag="pwst")
            for g in range(NGRP):
                acc = ps_pw.tile([128, cap], dt.float32, tag="pwacc")
                for h in range(HC):
                    for j in range(4):
                        c = 4 * g + j
                        nc.tensor.matmul(
                            acc[32 * j:32 * (j + 1), :],
                            wwt_s[h][:, c * K:(c + 1) * K],
                            xgt_s[h][:, c * cap:(c + 1) * cap],
                            start=(h == 0),
                            stop=(h == HC - 1),
                            tile_position=(0, 32 * j),
                        )
                nc.vector.tensor_copy(pwst[:, g * cap:(g + 1) * cap], acc[:])
            nc.scalar.dma_start(pw[:, :], pwst[:])

    nc.compile()
    return nc


def _prepare(x, cls, W_cls, W_words):
    """Host-side routing + layout prep. Returns (in_maps, meta)."""
    cls = np.asarray(cls).astype(np.int64)
    xb = np.asarray(x).astype(BF16)
    xbT = np.ascontiguousarray(xb.T)                      # [H, N]

    counts = np.bincount(cls, minlength=C)
    cap = int(max(8, counts.max()))
    cap = (cap + 3) // 4 * 4
    assert cap <= 44, f"class capacity {cap} exceeds kernel limit"

    order = np.argsort(cls, kind="stable")
    starts = np.zeros(C, np.int64)
    np.cumsum(counts[:-1], out=starts[1:])
    slots = np.arange(N, dtype=np.int64) - np.repeat(starts, counts)
    sel = np.zeros((C, cap), np.int64)
    sel[cls[order], slots] = order
    valid = np.arange(cap)[None, :] < counts[:, None]     # [C, cap]

    wcT = np.ascontiguousarray(W_cls.astype(BF16).T).reshape(HC, 128, C)

    in_maps = []
    for i in range(NCORES):
        xt_i = np.ascontiguousarray(xbT[:, i * TOK:(i + 1) * TOK]).reshape(
            HC, 128, TOK
        )
        idx = sel[i * CS:(i + 1) * CS].ravel()
        xgt_i = np.ascontiguousarray(xbT[:, idx]).reshape(HC, 128, CS * cap)
        ww_i = W_words[i * CS:(i + 1) * CS].astype(BF16)   # [CS, K, H]
        wwt_i = np.ascontiguousarray(ww_i.transpose(2, 0, 1)).reshape(
            HC, 128, CS * K
        )
        in_maps.append({"xt": xt_i, "wct": wcT, "wwt": wwt_i, "xgt": xgt_i})
    return in_maps, (cap, sel, valid)


def _assemble(results, meta, cls, b_cls, b_words):
    cap, sel, valid = meta
    p_class = np.concatenate([results[i]["pc"] for i in range(NCORES)], axis=0)
    p_class = p_class + np.asarray(b_cls)[None, :].astype(np.float32)

    # pw[32*j + w, g*cap + t] = logits for class 4g+j, slot t, word w
    full = np.empty((C, cap, K), np.float32)
    for i in range(NCORES):
        blk = results[i]["pw"].reshape(4, K, NGRP, cap)     # [j, w, g, t]
        full[i * CS:(i + 1) * CS] = (
            blk.transpose(2, 0, 3, 1).reshape(CS, cap, K)
        )
    p_words = np.empty((N, K), np.float32)
    p_words[sel[valid]] = full[valid]
    p_words = p_words + np.asarray(b_words).astype(np.float32)[np.asarray(cls)]
    return p_class, p_words


def run(inputs, trace=False, trace_kwargs=None):
    """Run the SPMD kernel on 8 cores. Returns ((p_class, p_words), results)."""
    x = np.asarray(inputs["x"])
    cls = np.asarray(inputs["cls"])
    in_maps, meta = _prepare(x, cls, inputs["W_cls"], inputs["W_words"])
    cap = meta[0]
    if cap not in _cache:
        _cache[cap] = _build(cap)
    nc = _cache[cap]
    res = run_bass_kernel_spmd(
        nc,
        in_maps,
        list(range(NCORES)),
        trace=trace,
        **(trace_kwargs or {}),
    )
    out = _assemble(res.results, meta, cls, inputs["b_cls"], inputs["b_words"])
    return out, res


def kernel(**inputs):
    (p_class, p_words), _ = run(inputs)
    return p_class, p_words


# revision 6
# speedup vs baseline: 1.1123x; 1.1123x over previous
"""Trainium2 Bass kernel for nn_ClassBasedSMDecoder.

Reference computation (N=8192 tokens, H=1024 hid, C=1024 classes, K=32):
    p_class = x @ W_cls.T + b_cls                      # [N, C]
    p_words = einsum('nh,nkh->nk', x, W_words[cls]) + b_words[cls]   # [N, K]

Sharding over 8 NeuronCores:
  * p_class: data-parallel over tokens — core i computes tokens
    [i*1024, (i+1)*1024) against the full (replicated) W_cls.
  * p_words: expert-parallel — core i owns classes [i*128, (i+1)*128).
    The host routes tokens to their class's core (sorted by class and
    padded to a fixed per-class capacity), each core runs per-class
    [cap, H] @ [H, K] matmuls with 4 classes packed into the 128-wide
    PE array via column tiling, and the host scatters results back.

All matmul inputs are cast to bf16 on the host (fp32 accumulate in PSUM).
"""

import numpy as np
import ml_dtypes

import concourse.bass as bass
import concourse.mybir as mybir
import concourse.tile as tile
from concourse import bacc
from concourse.bass_utils import run_bass_kernel_spmd

BF16 = ml_dtypes.bfloat16

N, H, C, K = 8192, 1024, 1024, 32
NCORES = 8
CS = C // NCORES        # 128 classes per core
TOK = N // NCORES       # 1024 tokens per core (p_class shard)
HC = H // 128           # 8 contraction chunks
NGRP = CS // 4          # 32 groups of 4 classes (column tiling)

_cache: dict = {}


def _build(cap: int):
    """Build + compile the per-core Bass program for class capacity `cap`."""
    G = CS * cap  # routed-token columns per core (padded)
    dt = mybir.dt
    nc = bacc.Bacc(
        "TRN2", target_bir_lowering=False, debug=False, enable_asserts=False
    )

    xt = nc.dram_tensor("xt", [HC, 128, TOK], dt.bfloat16, kind="ExternalInput")
    wct = nc.dram_tensor("wct", [HC, 128, C], dt.bfloat16, kind="ExternalInput")
    wwt = nc.dram_tensor("wwt", [HC, 128, CS * K], dt.bfloat16, kind="ExternalInput")
    xgt = nc.dram_tensor("xgt", [HC, 128, G], dt.bfloat16, kind="ExternalInput")
    pc = nc.dram_tensor("pc", [TOK, C], dt.bfloat16, kind="ExternalOutput")
    pw = nc.dram_tensor("pw", [128, NGRP * cap], dt.float32, kind="ExternalOutput")

    with tile.TileContext(nc) as tc:
        with (
            tc.tile_pool(name="big", bufs=1) as big,
            tc.tile_pool(name="stage", bufs=4) as stage,
            tc.tile_pool(name="ps_pc", bufs=4, space=bass.MemorySpace.PSUM) as ps_pc,
            tc.tile_pool(name="ps_pw", bufs=4, space=bass.MemorySpace.PSUM) as ps_pw,
        ):
            xt_s = [big.tile([128, TOK], dt.bfloat16, name=f"xt{h}", tag=f"xt{h}") for h in range(HC)]
            wct_s = [big.tile([128, C], dt.bfloat16, name=f"wct{h}", tag=f"wct{h}") for h in range(HC)]
            wwt_s = [
                big.tile([128, CS * K], dt.bfloat16, name=f"wwt{h}", tag=f"wwt{h}") for h in range(HC)
            ]
            xgt_s = [big.tile([128, G], dt.bfloat16, name=f"xgt{h}", tag=f"xgt{h}") for h in range(HC)]

            # Loads needed first by p_class, then the p_words operands.
            for h in range(HC):
                nc.sync.dma_start(xt_s[h][:], xt[h])
                nc.sync.dma_start(wct_s[h][:], wct[h])
            for h in range(HC):
                nc.sync.dma_start(wwt_s[h][:], wwt[h])
                nc.sync.dma_start(xgt_s[h][:], xgt[h])

            # ---- p_class: out[tok, cls] = sum_h x.T[h, tok] * W_cls.T[h, cls]
            for mt in range(TOK // 128):       # 8 token tiles (stationary)
                for ct in range(C // 512):     # 2 class column tiles (moving)
                    acc = ps_pc.tile([128, 512], dt.float32, tag="pcacc")
                    for h in range(HC):
                        nc.tensor.matmul(
                            acc[:],
                            xt_s[h][:, mt * 128:(mt + 1) * 128],
                            wct_s[h][:, ct * 512:(ct + 1) * 512],
                            start=(h == 0),
                            stop=(h == HC - 1),
                        )
                    st = stage.tile([128, 512], dt.bfloat16, tag="pcst")
                    nc.vector.tensor_copy(st[:], acc[:])
                    nc.scalar.dma_start(
                        pc[mt * 128:(mt + 1) * 128, ct * 512:(ct + 1) * 512], st[:]
                    )

            # ---- p_words: per class c, out[w, t] = sum_h Ww.T[h, c, w] * xg.T[h, t]
            # One full-width stationary load per (group of 4 classes, h-chunk):
            # lhsT = 4*K=128 contiguous word columns (bf16 FWL), moving = the
            # group's 4*cap routed-token columns.  Computes a [128, 4cap] block
            # of which the 4 diagonal [32, cap] class blocks are extracted.
            pwst = big.tile([128, NGRP * cap], dt.float32, tag="pwst")
            for g in range(NGRP):
                acc = ps_pw.tile([128, 4 * cap], dt.float32, tag="pwacc")
                for h in range(HC):
                    nc.tensor.matmul(
                        acc[:],
                        wwt_s[h][:, g * 4 * K:(g + 1) * 4 * K],
                        xgt_s[h][:, g * 4 * cap:(g + 1) * 4 * cap],
                        start=(h == 0),
                        stop=(h == HC - 1),
                    )
                for j in range(4):
                    nc.vector.tensor_copy(
                        pwst[32 * j:32 * (j + 1), g * cap:(g + 1) * cap],
                        acc[32 * j:32 * (j + 1), j * cap:(j + 1) * cap],
                    )
            nc.scalar.dma_start(pw[:, :], pwst[:])

    nc.compile()
    return nc


def _prepare(x, cls, W_cls, W_words):
    """Host-side routing + layout prep. Returns (in_maps, meta)."""
    cls = np.asarray(cls).astype(np.int64)
    xb = np.asarray(x).astype(BF16)
    xbT = np.ascontiguousarray(xb.T)                      # [H, N]

    counts = np.bincount(cls, minlength=C)
    cap = int(max(8, counts.max()))
    cap = (cap + 3) // 4 * 4
    assert cap <= 44, f"class capacity {cap} exceeds kernel limit"

    order = np.argsort(cls, kind="stable")
    starts = np.zeros(C, np.int64)
    np.cumsum(counts[:-1], out=starts[1:])
    slots = np.arange(N, dtype=np.int64) - np.repeat(starts, counts)
    sel = np.zeros((C, cap), np.int64)
    sel[cls[order], slots] = order
    valid = np.arange(cap)[None, :] < counts[:, None]     # [C, cap]

    wcT = np.ascontiguousarray(W_cls.astype(BF16).T).reshape(HC, 128, C)

    in_maps = []
    for i in range(NCORES):
        xt_i = np.ascontiguousarray(xbT[:, i * TOK:(i + 1) * TOK]).reshape(
            HC, 128, TOK
        )
        idx = sel[i * CS:(i + 1) * CS].ravel()
        xgt_i = np.ascontiguousarray(xbT[:, idx]).reshape(HC, 128, CS * cap)
        ww_i = W_words[i * CS:(i + 1) * CS].astype(BF16)   # [CS, K, H]
        wwt_i = np.ascontiguousarray(ww_i.transpose(2, 0, 1)).reshape(
            HC, 128, CS * K
        )
        in_maps.append({"xt": xt_i, "wct": wcT, "wwt": wwt_i, "xgt": xgt_i})
    return in_maps, (cap, sel, valid)


def _assemble(results, meta, cls, b_cls, b_words):
    cap, sel, valid = meta
    p_class = np.concatenate(
        [results[i]["pc"].astype(np.float32) for i in range(NCORES)], axis=0
    )
    p_class = p_class + np.asarray(b_cls)[None, :].astype(np.float32)

    # pw[32*j + w, g*cap + t] = logits for class 4g+j, slot t, word w
    full = np.empty((C, cap, K), np.float32)
    for i in range(NCORES):
        blk = results[i]["pw"].reshape(4, K, NGRP, cap)     # [j, w, g, t]
        full[i * CS:(i + 1) * CS] = (
            blk.transpose(2, 0, 3, 1).reshape(CS, cap, K)
        )
    p_words = np.empty((N, K), np.float32)
    p_words[sel[valid]] = full[valid]
    p_words = p_words + np.asarray(b_words).astype(np.float32)[np.asarray(cls)]
    return p_class, p_words


def run(inputs, trace=False, trace_kwargs=None):
    """Run the SPMD kernel on 8 cores. Returns ((p_class, p_words), results)."""
    x = np.asarray(inputs["x"])
    cls = np.asarray(inputs["cls"])
    in_maps, meta = _prepare(x, cls, inputs["W_cls"], inputs["W_words"])
    cap = meta[0]
    if cap not in _cache:
        _cache[cap] = _build(cap)
    nc = _cache[cap]
    res = run_bass_kernel_spmd(
        nc,
        in_maps,
        list(range(NCORES)),
        trace=trace,
        **(trace_kwargs or {}),
    )
    out = _assemble(res.results, meta, cls, inputs["b_cls"], inputs["b_words"])
    return out, res


def kernel(**inputs):
    (p_class, p_words), _ = run(inputs)
    return p_class, p_words


# revision 9
# speedup vs baseline: 1.7937x; 1.6126x over previous
"""Trainium2 Bass kernel for nn_ClassBasedSMDecoder.

Reference computation (N=8192 tokens, H=1024 hid, C=1024 classes, K=32):
    p_class = x @ W_cls.T + b_cls                      # [N, C]
    p_words = einsum('nh,nkh->nk', x, W_words[cls]) + b_words[cls]   # [N, K]

Sharding over 8 NeuronCores:
  * p_class: data-parallel over tokens — core i computes tokens
    [i*1024, (i+1)*1024) against the full (replicated) W_cls.
  * p_words: expert-parallel — core i owns classes [i*128, (i+1)*128).
    The host routes tokens to their class's core. Classes are sorted by
    token count (descending) per core, grouped 4 per PE pass with a
    per-group capacity (max count over the group across all cores), and
    4 groups per "block" (the DMA/scheduling unit). Each PE pass does
    one full-width stationary load (4 classes x 32 words = 128 columns)
    and streams the group's 4*cap routed-token columns, computing a
    [128, 4*cap] PSUM block whose 4 diagonal [32, cap] sub-blocks are
    the wanted logits (host discards the off-diagonal waste).

All matmul inputs are cast to bf16 on the host (fp32 accumulate in PSUM);
p_class is returned from the device in bf16.

DRAM layouts are partition-major: [128, ...] with the contraction chunk
index folded into the free dimension, so every tensor (or class-block)
loads with a single large contiguous DMA.
"""

import numpy as np
import ml_dtypes

import concourse.bass as bass
import concourse.mybir as mybir
import concourse.tile as tile
from concourse import bacc
from concourse.bass_utils import run_bass_kernel_spmd

BF16 = ml_dtypes.bfloat16

N, H, C, K = 8192, 1024, 1024, 32
NCORES = 8
CS = C // NCORES        # 128 classes per core
TOK = N // NCORES       # 1024 tokens per core (p_class shard)
HC = H // 128           # 8 contraction chunks
NGRP = CS // 4          # 32 groups of 4 classes
NBLK = 8                # 4 groups per block
GPB = NGRP // NBLK      # groups per block

_cache: dict = {}


def _build(caps: tuple):
    """Build + compile the per-core Bass program for group capacities `caps`."""
    caps = list(caps)
    assert len(caps) == NGRP
    gws = [4 * c for c in caps]              # group widths (tokens)
    capsum = sum(caps)
    gw_off = np.concatenate([[0], np.cumsum(gws)])   # within full token space
    bws = [sum(gws[b * GPB:(b + 1) * GPB]) for b in range(NBLK)]

    dt = mybir.dt
    nc = bacc.Bacc(
        "TRN2", target_bir_lowering=False, debug=False, enable_asserts=False
    )

    xt = nc.dram_tensor("xt", [128, HC * TOK], dt.bfloat16, kind="ExternalInput")
    wct = nc.dram_tensor("wct", [128, HC * C], dt.bfloat16, kind="ExternalInput")
    wwt = nc.dram_tensor(
        "wwt", [128, NBLK * HC * GPB * 128], dt.bfloat16, kind="ExternalInput"
    )
    xgt = nc.dram_tensor("xgt", [128, HC * 4 * capsum], dt.bfloat16,
                         kind="ExternalInput")
    pc = nc.dram_tensor("pc", [TOK, C], dt.bfloat16, kind="ExternalOutput")
    pw = nc.dram_tensor("pw", [128, 4 * capsum], dt.float32, kind="ExternalOutput")

    # sanity: SBUF per-partition budget (bytes)
    sbuf_bytes = (HC * TOK + HC * C + NBLK * HC * GPB * 128 + HC * 4 * capsum) * 2 \
        + 4 * capsum * 4 + 4 * 512 * 2
    assert sbuf_bytes < 190 * 1024, f"SBUF budget exceeded: {sbuf_bytes}"

    with tile.TileContext(nc) as tc:
        with (
            tc.tile_pool(name="big", bufs=1) as big,
            tc.tile_pool(name="stage", bufs=4) as stage,
            tc.tile_pool(name="ps_pc", bufs=4, space=bass.MemorySpace.PSUM) as ps_pc,
            tc.tile_pool(name="ps_pw", bufs=4, space=bass.MemorySpace.PSUM) as ps_pw,
        ):
            xt_s = big.tile([128, HC * TOK], dt.bfloat16, name="xt_s")
            wct_s = big.tile([128, HC * C], dt.bfloat16, name="wct_s")
            wwt_s = [
                big.tile([128, HC * GPB * 128], dt.bfloat16, name=f"wwt_s{b}",
                         tag=f"wwt_s{b}")
                for b in range(NBLK)
            ]
            xgt_s = [
                big.tile([128, HC * bws[b]], dt.bfloat16, name=f"xgt_s{b}",
                         tag=f"xgt_s{b}")
                for b in range(NBLK)
            ]
            pwst = big.tile([128, 4 * capsum], dt.float32, name="pwst")

            # Loads, in consumption order, each one large contiguous DMA.
            nc.sync.dma_start(xt_s[:], xt[:])
            nc.sync.dma_start(wct_s[:], wct[:])
            xgt_doff = [0]
            for b in range(NBLK):
                nc.sync.dma_start(
                    wwt_s[b][:],
                    wwt[:, b * HC * GPB * 128:(b + 1) * HC * GPB * 128],
                )
                nc.sync.dma_start(
                    xgt_s[b][:], xgt[:, xgt_doff[-1]:xgt_doff[-1] + HC * bws[b]]
                )
                xgt_doff.append(xgt_doff[-1] + HC * bws[b])

            def pc_tile(mt, ct):
                acc = ps_pc.tile([128, 512], dt.float32, tag="pcacc")
                for h in range(HC):
                    nc.tensor.matmul(
                        acc[:],
                        xt_s[:, h * TOK + mt * 128:h * TOK + (mt + 1) * 128],
                        wct_s[:, h * C + ct * 512:h * C + (ct + 1) * 512],
                        start=(h == 0),
                        stop=(h == HC - 1),
                    )
                st = stage.tile([128, 512], dt.bfloat16, tag="pcst")
                nc.vector.tensor_copy(st[:], acc[:])
                nc.scalar.dma_start(
                    pc[mt * 128:(mt + 1) * 128, ct * 512:(ct + 1) * 512], st[:]
                )

            def pw_block(b):
                for gl in range(GPB):
                    g = b * GPB + gl
                    gw = gws[g]
                    goff = gw_off[g] - gw_off[b * GPB]   # within block
                    acc = ps_pw.tile([128, gw], dt.float32, tag="pwacc")
                    for h in range(HC):
                        nc.tensor.matmul(
                            acc[:],
                            wwt_s[b][:, h * GPB * 128 + gl * 128:
                                      h * GPB * 128 + (gl + 1) * 128],
                            xgt_s[b][:, h * bws[b] + goff:h * bws[b] + goff + gw],
                            start=(h == 0),
                            stop=(h == HC - 1),
                        )
                    nc.vector.tensor_copy(
                        pwst[:, gw_off[g]:gw_off[g] + gw], acc[:]
                    )

            # Interleave: 2 p_class tiles up front, then alternate so each
            # p_words block runs right after its data lands.
            pc_units = [(mt, ct) for mt in range(TOK // 128) for ct in range(2)]
            sched = []
            pcs = iter(pc_units)
            for _ in range(4):
                sched.append(("pc", next(pcs)))
            for b in range(NBLK):
                sched.append(("pw", b))
                for _ in range(len(pc_units) // NBLK - (1 if b < 2 else 0)):
                    nxt = next(pcs, None)
                    if nxt is not None:
                        sched.append(("pc", nxt))
            for u in pcs:
                sched.append(("pc", u))
            for kind, arg in sched:
                if kind == "pc":
                    pc_tile(*arg)
                else:
                    pw_block(arg)

            nc.scalar.dma_start(pw[:, :], pwst[:])

    nc.compile()
    return nc


def _part_major(a, f):
    """[H, F] -> [128, HC*F] with chunk h at cols [h*F, (h+1)*F)."""
    return np.ascontiguousarray(
        a.reshape(HC, 128, f).transpose(1, 0, 2).reshape(128, HC * f)
    )


def _plan(cls):
    """Routing plan: per-core class sort, group capacities, token selection."""
    counts = np.bincount(cls, minlength=C).reshape(NCORES, CS)
    perm = np.argsort(-counts, axis=1, kind="stable")        # rank -> local class
    sc = np.take_along_axis(counts, perm, 1)                 # sorted counts desc
    caps = sc[:, ::4].max(axis=0)                            # [NGRP]
    caps = np.maximum(caps + (caps & 1), 2).astype(np.int64)  # even, >= 2
    # token lists per class
    order = np.argsort(cls, kind="stable")
    flat_counts = counts.reshape(-1)
    starts = np.zeros(C, np.int64)
    np.cumsum(flat_counts[:-1], out=starts[1:])
    return counts, perm, caps, order, starts


def _prepare(x, cls, W_cls, W_words):
    cls = np.asarray(cls).astype(np.int64)
    xb = np.asarray(x).astype(BF16)
    xbT = np.ascontiguousarray(xb.T)                         # [H, N]
    counts, perm, caps, order, starts = _plan(cls)
    capsum = int(caps.sum())

    wcT = _part_major(np.ascontiguousarray(W_cls.astype(BF16).T), C)

    wwb = W_words.astype(BF16)                               # [C, K, H]

    in_maps = []
    sels = []
    for i in range(NCORES):
        xt_i = _part_major(
            np.ascontiguousarray(xbT[:, i * TOK:(i + 1) * TOK]), TOK
        )
        # routed tokens, sorted-class order, padded per group capacity
        sel_i = np.zeros((CS, int(caps.max())), np.int64)
        valid_i = np.zeros((CS, int(caps.max())), bool)
        tok_cols = np.zeros(4 * capsum, np.int64)
        colpos = 0
        for r in range(CS):
            c_local = perm[i, r]
            c_glob = i * CS + c_local
            cnt = counts[i, c_local]
            cap = int(caps[r // 4])
            toks = order[starts[c_glob]:starts[c_glob] + cnt]
            sel_i[r, :cnt] = toks
            valid_i[r, :cnt] = True
            tok_cols[colpos:colpos + cnt] = toks
            colpos += cap
        sels.append((sel_i, valid_i))

        # xgt: per block, [H, bw] -> [128, HC*bw], concat blocks
        gws = 4 * caps
        bw_split = np.concatenate([[0], np.cumsum(gws.reshape(NBLK, GPB).sum(1))])
        parts = []
        for b in range(NBLK):
            lo, hi = int(bw_split[b]), int(bw_split[b + 1])
            parts.append(_part_major(
                np.ascontiguousarray(xbT[:, tok_cols[lo:hi]]), hi - lo
            ))
        xgt_i = np.concatenate(parts, axis=1)

        # wwt: per block, classes in sorted order
        parts = []
        for b in range(NBLK):
            ranks = perm[i, b * GPB * 4:(b + 1) * GPB * 4]
            Wb = wwb[i * CS + ranks]                          # [16, K, H]
            arr = np.ascontiguousarray(
                Wb.transpose(2, 0, 1).reshape(H, GPB * 4 * K)
            )
            parts.append(_part_major(arr, GPB * 4 * K))
        wwt_i = np.concatenate(parts, axis=1)

        in_maps.append({"xt": xt_i, "wct": wcT, "wwt": wwt_i, "xgt": xgt_i})
    return in_maps, (caps, perm, sels)


def _assemble(results, meta, cls, b_cls, b_words):
    caps, perm, sels = meta
    p_class = np.concatenate(
        [results[i]["pc"].astype(np.float32) for i in range(NCORES)], axis=0
    )
    p_class = p_class + np.asarray(b_cls)[None, :].astype(np.float32)

    gw_off = np.concatenate([[0], np.cumsum(4 * caps)])
    p_words = np.empty((N, K), np.float32)
    for i in range(NCORES):
        pw_i = results[i]["pw"]                               # [128, 4*capsum]
        sel_i, valid_i = sels[i]
        for r in range(CS):
            g, j = r // 4, r % 4
            cap = int(caps[g])
            nv = int(valid_i[r].sum())
            if nv == 0:
                continue
            base = int(gw_off[g]) + j * cap
            blk = pw_i[32 * j:32 * (j + 1), base:base + nv]    # [K, nv]
            p_words[sel_i[r, :nv]] = blk.T
    p_words = p_words + np.asarray(b_words).astype(np.float32)[np.asarray(cls)]
    return p_class, p_words


def run(inputs, trace=False, trace_kwargs=None):
    """Run the SPMD kernel on 8 cores. Returns ((p_class, p_words), results)."""
    x = np.asarray(inputs["x"])
    cls = np.asarray(inputs["cls"])
    in_maps, meta = _prepare(x, cls, inputs["W_cls"], inputs["W_words"])
    key = tuple(int(c) for c in meta[0])
    if key not in _cache:
        _cache[key] = _build(key)
    nc = _cache[key]
    res = run_bass_kernel_spmd(
        nc,
        in_maps,
        list(range(NCORES)),
        trace=trace,
        **(trace_kwargs or {}),
    )
    out = _assemble(res.results, meta, cls, inputs["b_cls"], inputs["b_words"])
    return out, res


def kernel(**inputs):
    (p_class, p_words), _ = run(inputs)
    return p_class, p_words
